# revision 25
# baseline (speedup 1.0000x reference)
"""DETR loss (cost matrix + Hungarian matching + losses) on 8 Trainium2 cores.

Sharding: data-parallel over batch. Each core handles 4 images as 2 pairs of 2
images packed into 128 SBUF partitions (2 images x 64 targets). The device
computes, per image, the [T=64, Q=300] matching-cost block (L1 cdist + class
cost + pairwise GIoU cost). The inherently serial Hungarian assignment runs on
host (exactly as in the reference, whose matcher is host-side numpy), and the
scalar loss is assembled on host from the matched pairs.
"""
import numpy as np

B, Q, T, C = 32, 300, 64, 2
N_CORES = 8
IMGS_PER_CORE = B // N_CORES          # 4
PAIRS_PER_CORE = IMGS_PER_CORE // 2   # 2
CLS_SCALE = 0.1
BBOX_SCALE = 5.0
GIOU_SCALE = 2.0

PIPE_DT = "bfloat16"   # dtype of the post-PSUM cost pipeline

# engine assignment knobs (tuned via CoreSim cost model)
R_ENGS = ["scalar", "scalar", "vector", "vector"]   # r1..r4
B_ENGS = ["scalar", "scalar", "scalar", "scalar"]   # b1..b4
TT_ENG = "gpsimd"    # LB / P2 / OUT adds
WE_ENG = "vector"

_CACHE = {}


def _split_wide_waits(nc, mybir, max_waits=1):
    """This walrus rejects instructions carrying >1 sem-wait; hoist extra
    waits onto NoOp carriers inserted just before (same engine, in-order)."""
    n_new = 0
    for bb in nc.main_func.blocks:
        insts = bb.instructions
        i = 0
        while i < len(insts):
            ins = insts[i]
            si = ins.sync_info
            if (
                si is not None
                and si.on_wait is not None
                and len(si.on_wait) > max_waits
            ):
                waits = list(si.on_wait)
                si.on_wait = waits[:max_waits]
                extra = waits[max_waits:]
                for j in range(0, len(extra), max_waits):
                    nd = mybir.InstNoOp(name=f"{ins.name}-xw{n_new}", ins=[], outs=[])
                    nd.engine = ins.engine
                    nd.sync_info = mybir.SyncInfo(
                        on_wait=extra[j : j + max_waits], on_update=[]
                    )
                    nc.register_instruction(nd, overwrite=True)
                    insts.insert(i, nd)
                    n_new += 1
                    i += 1
            i += 1
    return n_new


def _build_program():
    import concourse.bass as bass
    import concourse.mybir as mybir
    from concourse.tile import TileContext

    f32 = mybir.dt.float32
    DT = getattr(mybir.dt, PIPE_DT)
    op = mybir.AluOpType
    AF = mybir.ActivationFunctionType
    # qrows slots: px1, -px2, py1, -py2 | pcx, pcy, pw, ph | area1, f
    NQROW = 10
    # trows: ty1, nty2, tw, th, area2, ntx1, tx2, ntcx, ntcy, ntw, nth
    NTROW = 11
    QW3 = NQROW * Q

    bf16 = mybir.dt.bfloat16
    nc = bass.Bass()
    # per pair: 3 groups x 2 imgs x 4 quantity-slots of Q cols
    qrows = nc.declare_dram_parameter("qrows", [PAIRS_PER_CORE, 96, 4 * Q], bf16, isOutput=False)
    trows = nc.declare_dram_parameter("trows", [128, PAIRS_PER_CORE * NTROW], f32, isOutput=False)
    cost_o = nc.declare_dram_parameter("cost", [PAIRS_PER_CORE, 128, Q], DT, isOutput=True)

    with TileContext(nc) as tc:
        with (
            nc.allow_low_precision(reason="bf16 cost pipeline; assignment-tolerant"),
            tc.tile_pool(name="const", bufs=1) as cpool,
            tc.tile_pool(name="sb", bufs=2) as sb,
            tc.tile_pool(name="ps", bufs=4, space="PSUM") as ps,
        ):
            # indicator built on-chip at each legal matmul base (0/32/64):
            # row0 = [1]*64+[0]*64 (applied to A-B), row1 = all ones (applied to B)
            indt = cpool.tile([96, 128], bf16)
            for g in range(3):
                nc.vector.memset(indt[g * 32:g * 32 + 2, :], 1.0)
                nc.vector.memset(indt[g * 32:g * 32 + 1, 64:128], 0.0)

            # per-pair input DMA into partition groups at legal matmul bases
            qts = []
            for p in range(PAIRS_PER_CORE):
                qt = sb.tile([96, 4 * Q], bf16, tag=f"qt{p}")
                (nc.sync if p % 2 == 0 else nc.gpsimd).dma_start(out=qt[:], in_=qrows[p])
                qts.append(qt)
            trt = sb.tile([128, PAIRS_PER_CORE * NTROW], f32, tag="trt")
            nc.scalar.dma_start(out=trt[:], in_=trows[:])

            def mm_round(p, ks):
                Mr = ps.tile([128, 2 * 512], f32, tag="mega")
                Mrv = Mr[:].rearrange("p (s k) -> p s k", k=512)
                for i, k in enumerate(ks):
                    g, ck = (0, k) if k < 4 else ((1, k - 4) if k < 7 else (2, k - 7))
                    nc.tensor.matmul(Mrv[:, i, 0:Q], lhsT=indt[g * 32:g * 32 + 2, :],
                                     rhs=qts[p][g * 32:g * 32 + 2, ck * Q:(ck + 1) * Q],
                                     start=True, stop=True)
                return Mrv

            def fused(out_ap, psum_ap, bias_ap, kind, eng):
                if eng == "scalar":
                    nc.scalar.activation(out_ap, psum_ap,
                                         AF.Relu if kind == "relu" else AF.Abs,
                                         bias=bias_ap)
                else:
                    getattr(nc, eng).tensor_scalar(
                        out=out_ap, in0=psum_ap, scalar1=bias_ap, scalar2=0.0,
                        op0=op.add,
                        op1=op.max if kind == "relu" else op.abs_max)

            st = [dict() for _ in range(PAIRS_PER_CORE)]
            for p in range(PAIRS_PER_CORE):
                st[p]["Mx"] = mm_round(p, [0, 1])      # px1, -px2
                st[p]["My"] = mm_round(p, [2, 3])      # py1, -py2
            for p in range(PAIRS_PER_CORE):
                def sc(k, p=p):
                    return trt[:, p * NTROW + k:p * NTROW + k + 1]
                R13 = sb.tile([128, 2 * Q], DT, tag=f"R13_{p}")
                R24 = sb.tile([128, 2 * Q], DT, tag=f"R24_{p}")
                fused(R13[:, :Q], st[p]["Mx"][:, 0, 0:Q], sc(3), "relu", R_ENGS[0])
                fused(R24[:, :Q], st[p]["Mx"][:, 1, 0:Q], sc(4), "relu", R_ENGS[1])
                fused(R13[:, Q:], st[p]["My"][:, 0, 0:Q], sc(5), "relu", R_ENGS[2])
                fused(R24[:, Q:], st[p]["My"][:, 1, 0:Q], sc(6), "relu", R_ENGS[3])
                st[p]["R13"], st[p]["R24"] = R13, R24
                st[p]["Mc"] = mm_round(p, [4, 5])      # pcx, pcy
                st[p]["Mw"] = mm_round(p, [6, 7])      # pw, ph
            for p in range(PAIRS_PER_CORE):
                def sc(k, p=p):
                    return trt[:, p * NTROW + k:p * NTROW + k + 1]
                S = sb.tile([128, 2 * Q], DT, tag=f"S_{p}")
                nc.vector.tensor_tensor(out=S[:], in0=st[p]["R13"][:], in1=st[p]["R24"][:], op=op.add)
                st[p]["S"] = S
                B12 = sb.tile([128, 2 * Q], DT, tag=f"B12_{p}")
                B34 = sb.tile([128, 2 * Q], DT, tag=f"B34_{p}")
                fused(B12[:, :Q], st[p]["Mc"][:, 0, 0:Q], sc(7), "abs", B_ENGS[0])
                fused(B12[:, Q:], st[p]["Mc"][:, 1, 0:Q], sc(8), "abs", B_ENGS[1])
                fused(B34[:, :Q], st[p]["Mw"][:, 0, 0:Q], sc(9), "abs", B_ENGS[2])
                fused(B34[:, Q:], st[p]["Mw"][:, 1, 0:Q], sc(10), "abs", B_ENGS[3])
                st[p]["B12"], st[p]["B34"] = B12, B34
            for p in range(PAIRS_PER_CORE):
                def sc(k, p=p):
                    return trt[:, p * NTROW + k:p * NTROW + k + 1]
                tt_eng = getattr(nc, TT_ENG)
                S = st[p]["S"]
                NW = sb.tile([128, 2 * Q], DT, tag=f"NW_{p}")
                nc.vector.tensor_scalar(out=NW[:, :Q], in0=S[:, :Q], scalar1=sc(0), scalar2=0.0,
                                        op0=op.subtract, op1=op.min)
                nc.vector.tensor_scalar(out=NW[:, Q:], in0=S[:, Q:], scalar1=sc(1), scalar2=0.0,
                                        op0=op.subtract, op1=op.min)
                WE = sb.tile([128, 2 * Q], DT, tag=f"WE_{p}")
                getattr(nc, WE_ENG).tensor_tensor(
                    out=WE[:].rearrange("p (a b) -> p a b", b=Q),
                    in0=S[:].rearrange("p (a b) -> p a b", b=Q),
                    in1=st[p]["Mw"][:, :, 0:Q], op=op.add)
                LB = sb.tile([128, 2 * Q], DT, tag=f"LB_{p}")
                tt_eng.tensor_tensor(out=LB[:], in0=st[p]["B12"][:], in1=st[p]["B34"][:], op=op.add)
                st[p]["Ml"] = mm_round(p, [8, 9])      # area1, f
                T1 = sb.tile([128, 2 * Q], DT, tag=f"T1_{p}")   # [inter | -union]
                nc.vector.tensor_tensor(out=T1[:, :Q], in0=NW[:, :Q], in1=NW[:, Q:], op=op.mult)
                nc.vector.scalar_tensor_tensor(out=T1[:, Q:], in0=T1[:, :Q], scalar=sc(2),
                                               in1=st[p]["Ml"][:, 0, 0:Q], op0=op.subtract, op1=op.subtract)
                ENC = sb.tile([128, Q], DT, tag=f"ENC_{p}")
                nc.vector.tensor_tensor(out=ENC[:], in0=WE[:, :Q], in1=WE[:, Q:], op=op.mult)
                RC = sb.tile([128, 2 * Q], DT, tag=f"RC_{p}")   # [1/-union | 1/enc]
                nc.vector.reciprocal(out=RC[:, :Q], in_=T1[:, Q:])
                nc.vector.reciprocal(out=RC[:, Q:], in_=ENC[:])
                IU = sb.tile([128, 2 * Q], DT, tag=f"IU_{p}")   # [-iou | -ue]
                nc.vector.tensor_tensor(out=IU[:], in0=T1[:], in1=RC[:], op=op.mult)
                P1 = sb.tile([128, Q], DT, tag=f"P1_{p}")
                nc.vector.tensor_tensor(out=P1[:], in0=IU[:, :Q], in1=IU[:, Q:], op=op.add)
                P2 = sb.tile([128, Q], DT, tag=f"P2_{p}")
                tt_eng.tensor_tensor(out=P2[:], in0=LB[:, :Q], in1=LB[:, Q:], op=op.add)
                P3 = sb.tile([128, Q], DT, tag=f"P3_{p}")
                nc.vector.tensor_tensor(out=P3[:], in0=P2[:], in1=st[p]["Ml"][:, 1, 0:Q], op=op.add)
                OUT = sb.tile([128, Q], DT, tag=f"OUT_{p}")
                tt_eng.tensor_tensor(out=OUT[:], in0=P3[:], in1=P1[:], op=op.add)
                (nc.sync if p % 2 == 0 else nc.gpsimd).dma_start(out=cost_o[p], in_=OUT[:])

    _split_wide_waits(nc, mybir)
    return nc


def _lsa(cost):
    # Hungarian (shortest augmenting path), identical algorithm to reference.
    cost = np.asarray(cost, dtype=np.float64)
    n, m = cost.shape
    u = np.zeros(n + 1)
    v = np.zeros(m + 1)
    p = np.zeros(m + 1, dtype=np.int64)
    way = np.zeros(m + 1, dtype=np.int64)
    for i in range(1, n + 1):
        p[0] = i
        j0 = 0
        minv = np.full(m + 1, np.inf)
        used = np.zeros(m + 1, dtype=bool)
        while True:
            used[j0] = True
            i0 = p[j0]
            cur = cost[i0 - 1, :] - u[i0] - v[1:]
            free = ~used[1:]
            upd = free & (cur < minv[1:])
            minv[1:][upd] = cur[upd]
            way[1:][upd] = j0
            cand = np.where(free, minv[1:], np.inf)
            j1 = int(np.argmin(cand)) + 1
            delta = cand[j1 - 1]
            u[p[used]] += delta
            v[used] -= delta
            minv[~used] -= delta
            j0 = j1
            if p[j0] == 0:
                break
        while j0:
            j1 = way[j0]
            p[j0] = p[j1]
            j0 = j1
    ans = np.zeros(n, dtype=np.int64)
    for j in range(1, m + 1):
        if p[j] > 0:
            ans[p[j] - 1] = j - 1
    return ans


def _host_prep(logits, pred_bbox, target_bbox):
    import ml_dtypes
    logits = np.ascontiguousarray(logits, np.float32)
    pb = np.ascontiguousarray(pred_bbox, np.float32)
    tb = np.ascontiguousarray(target_bbox, np.float32)

    pcx, pcy, pw, ph = pb[..., 0], pb[..., 1], pb[..., 2], pb[..., 3]
    px1, py1 = pcx - 0.5 * pw, pcy - 0.5 * ph
    px2, py2 = pcx + 0.5 * pw, pcy + 0.5 * ph
    area1 = pw * ph
    dl = (logits[..., 1] - logits[..., 0]).astype(np.float64)
    f = (1.0 / (1.0 + np.exp(-dl))).astype(np.float32)   # 1 - p0 = sigmoid(l1-l0)
    # [B, 10, Q], quantity-major
    qr_all = np.stack([px1, -px2, py1, -py2, pcx, pcy, pw, ph, area1, f], axis=1)

    tcx, tcy, tw, th = tb[..., 0], tb[..., 1], tb[..., 2], tb[..., 3]
    tx1, ty1 = tcx - 0.5 * tw, tcy - 0.5 * th
    tx2, ty2 = tcx + 0.5 * tw, tcy + 0.5 * th
    area2 = tw * th
    # [B, T, 11]
    tr_all = np.stack([tw, th, area2, -tx1, tx2, -ty1, ty2, -tcx, -tcy, -tw, -th],
                      axis=-1)

    in_maps = []
    for c in range(N_CORES):
        i0 = c * IMGS_PER_CORE
        # qrows: [pair, group(3) x img(2), 4*Q] bf16, groups of quantities
        qc4 = qr_all[i0:i0 + IMGS_PER_CORE].reshape(PAIRS_PER_CORE, 2, 10, Q)
        # pre-round to bf16 so the A-B row is an exact difference of bf16 values
        qc4 = qc4.astype(ml_dtypes.bfloat16).astype(np.float32)
        qc = np.zeros((PAIRS_PER_CORE, 96, 4 * Q), np.float32)
        for g, ks in enumerate(([0, 1, 2, 3], [4, 5, 6], [7, 8, 9])):
            for j, k in enumerate(ks):
                # row0 = imgA - imgB (selected on partitions 0-63), row1 = imgB
                qc[:, g * 32 + 0, j * Q:(j + 1) * Q] = qc4[:, 0, k, :] - qc4[:, 1, k, :]
                qc[:, g * 32 + 1, j * Q:(j + 1) * Q] = qc4[:, 1, k, :]
        # trows: [128 partitions, pair*11]
        tc_ = tr_all[i0:i0 + IMGS_PER_CORE].reshape(PAIRS_PER_CORE, 128, 11)
        tc_ = tc_.transpose(1, 0, 2).reshape(128, PAIRS_PER_CORE * 11)
        in_maps.append({
            "qrows": np.ascontiguousarray(qc).astype(ml_dtypes.bfloat16),
            "trows": np.ascontiguousarray(tc_),
        })
    return in_maps


def _finalize(logits, pred_bbox, target_bbox, target_labels, src):
    labels = np.asarray(target_labels).astype(np.int64)
    lg = np.asarray(logits, np.float64)
    pb = np.asarray(pred_bbox, np.float64)
    tb = np.asarray(target_bbox, np.float64)
    bidx = np.arange(B)[:, None]

    # CE pieces (exact, host): nlpk = -logp_k
    dl = lg[..., 1] - lg[..., 0]
    nlp1 = np.logaddexp(0.0, -dl)       # -logp1 = softplus(l0-l1)
    nlp0 = np.logaddexp(0.0, dl)        # -logp0 = softplus(l1-l0)
    g = nlp0 - CLS_SCALE * nlp1         # matched-query correction (labels are 0)
    A = nlp1.sum()
    w = np.ones(C); w[-1] = CLS_SCALE
    wt_sum = CLS_SCALE * (B * Q) + np.sum(w[labels] - CLS_SCALE)
    ce = (CLS_SCALE * A + g[bidx, src].sum()) / wt_sum

    mp = pb[bidx, src].reshape(-1, 4)
    mt = tb.reshape(-1, 4)
    nb = B * T
    l1 = np.abs(mp - mt).sum() / nb

    def corners(x):
        cx, cy, ww, hh = x[:, 0], x[:, 1], x[:, 2], x[:, 3]
        return np.stack([cx - .5 * ww, cy - .5 * hh, cx + .5 * ww, cy + .5 * hh], -1)

    c1, c2 = corners(mp), corners(mt)
    a1 = (c1[:, 2] - c1[:, 0]) * (c1[:, 3] - c1[:, 1])
    a2 = (c2[:, 2] - c2[:, 0]) * (c2[:, 3] - c2[:, 1])
    lt = np.maximum(c1[:, :2], c2[:, :2]); rb = np.minimum(c1[:, 2:], c2[:, 2:])
    wh = np.clip(rb - lt, 0, None); inter = wh[:, 0] * wh[:, 1]
    union = a1 + a2 - inter
    iou = inter / union
    lte = np.minimum(c1[:, :2], c2[:, :2]); rbe = np.maximum(c1[:, 2:], c2[:, 2:])
    whe = np.clip(rbe - lte, 0, None); encl = whe[:, 0] * whe[:, 1]
    giou = iou - (encl - union) / encl
    lgi = (1.0 - giou).sum() / nb
    return ce + BBOX_SCALE * l1 + GIOU_SCALE * lgi


def kernel(logits, pred_bbox, target_bbox, target_labels):
    import os
    os.environ["BASS_NEVER_TRACE"] = "1"   # no NTFF hook in this container
    from concourse.bass_utils import run_bass_kernel_spmd

    if "nc" not in _CACHE:
        _CACHE["nc"] = _build_program()
    nc = _CACHE["nc"]

    in_maps = _host_prep(logits, pred_bbox, target_bbox)
    res = run_bass_kernel_spmd(nc, in_maps, core_ids=list(range(N_CORES)))
    _CACHE["last_res"] = res

    cost_T = np.zeros((B, T, Q), np.float32)   # [img, target, query]
    for c in range(N_CORES):
        cb = np.asarray(res.results[c]["cost"]).astype(np.float32).reshape(PAIRS_PER_CORE, 2, 64, Q)
        i0 = c * IMGS_PER_CORE
        for p in range(PAIRS_PER_CORE):
            cost_T[i0 + 2 * p] = cb[p, 0]
            cost_T[i0 + 2 * p + 1] = cb[p, 1]

    src = np.zeros((B, T), np.int64)
    for i in range(B):
        src[i] = _lsa(cost_T[i])

    total = _finalize(logits, pred_bbox, target_bbox, target_labels, src)
    return np.float32(total)


# revision 37
# speedup vs baseline: 1.1240x; 1.1240x over previous
"""DETR loss (cost matrix + Hungarian matching + losses) on 8 Trainium2 cores.

Sharding: data-parallel over batch. Each core handles 4 images as 2 pairs of 2
images packed into 128 SBUF partitions (2 images x 64 targets). The device
computes, per image, the [T=64, Q=300] matching-cost block (L1 cdist + class
cost + pairwise GIoU cost). The inherently serial Hungarian assignment runs on
host (exactly as in the reference, whose matcher is host-side numpy), and the
scalar loss is assembled on host from the matched pairs.
"""
import numpy as np

B, Q, T, C = 32, 300, 64, 2
N_CORES = 8
IMGS_PER_CORE = B // N_CORES          # 4
PAIRS_PER_CORE = IMGS_PER_CORE // 2   # 2
CLS_SCALE = 0.1
BBOX_SCALE = 5.0
GIOU_SCALE = 2.0

PIPE_DT = "bfloat16"   # dtype of the post-PSUM cost pipeline

# engine assignment knobs (tuned via CoreSim cost model)
R_ENGS = ["scalar", "scalar", "vector", "vector"]   # r1..r4
B_ENGS = ["scalar", "scalar", "scalar", "scalar"]   # b1..b4
TT_ENG = "gpsimd"    # LB / P2 / OUT adds
WE_ENG = "vector"

_CACHE = {}


def _split_wide_waits(nc, mybir, max_waits=1):
    """This walrus rejects instructions carrying >1 sem-wait; hoist extra
    waits onto NoOp carriers inserted just before (same engine, in-order)."""
    n_new = 0
    for bb in nc.main_func.blocks:
        insts = bb.instructions
        i = 0
        while i < len(insts):
            ins = insts[i]
            si = ins.sync_info
            if (
                si is not None
                and si.on_wait is not None
                and len(si.on_wait) > max_waits
            ):
                waits = list(si.on_wait)
                si.on_wait = waits[:max_waits]
                extra = waits[max_waits:]
                for j in range(0, len(extra), max_waits):
                    nd = mybir.InstNoOp(name=f"{ins.name}-xw{n_new}", ins=[], outs=[])
                    nd.engine = ins.engine
                    nd.sync_info = mybir.SyncInfo(
                        on_wait=extra[j : j + max_waits], on_update=[]
                    )
                    nc.register_instruction(nd, overwrite=True)
                    insts.insert(i, nd)
                    n_new += 1
                    i += 1
            i += 1
    return n_new


def _build_program():
    import concourse.bass as bass
    import concourse.mybir as mybir
    from concourse.tile import TileContext

    f32 = mybir.dt.float32
    DT = getattr(mybir.dt, PIPE_DT)
    op = mybir.AluOpType
    AF = mybir.ActivationFunctionType
    # qrows slots: px1, -px2, py1, -py2 | pcx, pcy, pw, ph | area1, f
    NQROW = 10
    # trows: ty1, nty2, tw, th, area2, ntx1, tx2, ntcx, ntcy, ntw, nth
    NTROW = 11
    QW3 = NQROW * Q

    bf16 = mybir.dt.bfloat16
    nc = bass.Bass()
    # per pair: 3 groups x 2 imgs x 4 quantity-slots of Q cols
    qrows = nc.declare_dram_parameter("qrows", [PAIRS_PER_CORE, 96, 4 * Q], bf16, isOutput=False)
    trows = nc.declare_dram_parameter("trows", [128, PAIRS_PER_CORE * NTROW], f32, isOutput=False)
    cost_o = nc.declare_dram_parameter("cost", [PAIRS_PER_CORE, 128, Q], DT, isOutput=True)

    with TileContext(nc) as tc:
        with (
            nc.allow_low_precision(reason="bf16 cost pipeline; assignment-tolerant"),
            tc.tile_pool(name="const", bufs=1) as cpool,
            tc.tile_pool(name="sb", bufs=2) as sb,
            tc.tile_pool(name="ps", bufs=4, space="PSUM") as ps,
        ):
            # indicator built on-chip at each legal matmul base (0/32/64):
            # row0 = [1]*64+[0]*64 (applied to A-B), row1 = all ones (applied to B)
            indt = cpool.tile([96, 128], bf16)
            for g in range(3):
                nc.vector.memset(indt[g * 32:g * 32 + 2, :], 1.0)
                nc.vector.memset(indt[g * 32:g * 32 + 1, 64:128], 0.0)
            # warm the ACT table set (Relu+Abs) while input DMAs are in flight
            warm = cpool.tile([2, 128], DT)
            nc.scalar.activation(warm[:], indt[0:2, :], AF.Relu)
            nc.scalar.activation(warm[:], indt[0:2, :], AF.Abs)

            # per-pair input DMA into partition groups at legal matmul bases
            qts = []
            for p in range(PAIRS_PER_CORE):
                qt = sb.tile([96, 4 * Q], bf16, tag=f"qt{p}")
                (nc.sync if p % 2 == 0 else nc.gpsimd).dma_start(out=qt[:], in_=qrows[p])
                qts.append(qt)
            trt = sb.tile([128, PAIRS_PER_CORE * NTROW], f32, tag="trt")
            nc.scalar.dma_start(out=trt[:], in_=trows[:])

            def mm_round(p, ks):
                Mr = ps.tile([128, 2 * 512], f32, tag="mega")
                Mrv = Mr[:].rearrange("p (s k) -> p s k", k=512)
                for i, k in enumerate(ks):
                    g, ck = (0, k) if k < 4 else ((1, k - 4) if k < 7 else (2, k - 7))
                    nc.tensor.matmul(Mrv[:, i, 0:Q], lhsT=indt[g * 32:g * 32 + 2, :],
                                     rhs=qts[p][g * 32:g * 32 + 2, ck * Q:(ck + 1) * Q],
                                     start=True, stop=True)
                return Mrv

            def fused(out_ap, psum_ap, bias_ap, kind, eng):
                if eng == "scalar":
                    nc.scalar.activation(out_ap, psum_ap,
                                         AF.Relu if kind == "relu" else AF.Abs,
                                         bias=bias_ap)
                else:
                    getattr(nc, eng).tensor_scalar(
                        out=out_ap, in0=psum_ap, scalar1=bias_ap, scalar2=0.0,
                        op0=op.add,
                        op1=op.max if kind == "relu" else op.abs_max)

            st = [dict() for _ in range(PAIRS_PER_CORE)]
            for p in range(PAIRS_PER_CORE):
                st[p]["Mx"] = mm_round(p, [0, 1])      # px1, -px2
                st[p]["My"] = mm_round(p, [2, 3])      # py1, -py2
            for p in range(PAIRS_PER_CORE):
                def sc(k, p=p):
                    return trt[:, p * NTROW + k:p * NTROW + k + 1]
                R13 = sb.tile([128, 2 * Q], DT, tag=f"R13_{p}")
                R24 = sb.tile([128, 2 * Q], DT, tag=f"R24_{p}")
                fused(R13[:, :Q], st[p]["Mx"][:, 0, 0:Q], sc(3), "relu", R_ENGS[0])
                fused(R24[:, :Q], st[p]["Mx"][:, 1, 0:Q], sc(4), "relu", R_ENGS[1])
                fused(R13[:, Q:], st[p]["My"][:, 0, 0:Q], sc(5), "relu", R_ENGS[2])
                fused(R24[:, Q:], st[p]["My"][:, 1, 0:Q], sc(6), "relu", R_ENGS[3])
                st[p]["R13"], st[p]["R24"] = R13, R24
                st[p]["Mc"] = mm_round(p, [4, 5])      # pcx, pcy
                st[p]["Mw"] = mm_round(p, [6, 7])      # pw, ph
            for p in range(PAIRS_PER_CORE):
                def sc(k, p=p):
                    return trt[:, p * NTROW + k:p * NTROW + k + 1]
                S = sb.tile([128, 2 * Q], DT, tag=f"S_{p}")
                (nc.vector if p % 2 == 0 else nc.gpsimd).tensor_tensor(
                    out=S[:], in0=st[p]["R13"][:], in1=st[p]["R24"][:], op=op.add)
                st[p]["S"] = S
                B12 = sb.tile([128, 2 * Q], DT, tag=f"B12_{p}")
                B34 = sb.tile([128, 2 * Q], DT, tag=f"B34_{p}")
                fused(B12[:, :Q], st[p]["Mc"][:, 0, 0:Q], sc(7), "abs", B_ENGS[0])
                fused(B12[:, Q:], st[p]["Mc"][:, 1, 0:Q], sc(8), "abs", B_ENGS[1])
                fused(B34[:, :Q], st[p]["Mw"][:, 0, 0:Q], sc(9), "abs", B_ENGS[2])
                fused(B34[:, Q:], st[p]["Mw"][:, 1, 0:Q], sc(10), "abs", B_ENGS[3])
                st[p]["B12"], st[p]["B34"] = B12, B34
            for p in range(PAIRS_PER_CORE):
                def sc(k, p=p):
                    return trt[:, p * NTROW + k:p * NTROW + k + 1]
                tt_eng = getattr(nc, TT_ENG)
                ve = nc.vector if p % 2 == 0 else nc.gpsimd   # alternate pairs across engines
                S = st[p]["S"]
                NW = sb.tile([128, 2 * Q], DT, tag=f"NW_{p}")
                nc.vector.tensor_scalar(out=NW[:, :Q], in0=S[:, :Q], scalar1=sc(0), scalar2=0.0,
                                        op0=op.subtract, op1=op.min)
                nc.vector.tensor_scalar(out=NW[:, Q:], in0=S[:, Q:], scalar1=sc(1), scalar2=0.0,
                                        op0=op.subtract, op1=op.min)
                WE = sb.tile([128, 2 * Q], DT, tag=f"WE_{p}")
                getattr(nc, WE_ENG).tensor_tensor(
                    out=WE[:].rearrange("p (a b) -> p a b", b=Q),
                    in0=S[:].rearrange("p (a b) -> p a b", b=Q),
                    in1=st[p]["Mw"][:, :, 0:Q], op=op.add)
                LB = sb.tile([128, 2 * Q], DT, tag=f"LB_{p}")
                tt_eng.tensor_tensor(out=LB[:], in0=st[p]["B12"][:], in1=st[p]["B34"][:], op=op.add)
                st[p]["Ml"] = mm_round(p, [8, 9])      # area1, f
                # area1|f to SBUF via ACT so tail ops can run off-PSUM on any engine
                FA = sb.tile([128, 2 * Q], DT, tag=f"FA_{p}")
                if p % 2 == 0:
                    nc.vector.tensor_copy(FA[:].rearrange("p (a b) -> p a b", b=Q),
                                          st[p]["Ml"][:, :, 0:Q])
                else:
                    nc.scalar.copy(out=FA[:].rearrange("p (a b) -> p a b", b=Q),
                                   in_=st[p]["Ml"][:, :, 0:Q])
                T1 = sb.tile([128, 2 * Q], DT, tag=f"T1_{p}")   # [inter | -union]
                ve.tensor_tensor(out=T1[:, :Q], in0=NW[:, :Q], in1=NW[:, Q:], op=op.mult)
                nc.vector.scalar_tensor_tensor(out=T1[:, Q:], in0=T1[:, :Q], scalar=sc(2),
                                               in1=FA[:, :Q], op0=op.subtract, op1=op.subtract)
                ENC = sb.tile([128, Q], DT, tag=f"ENC_{p}")
                ve.tensor_tensor(out=ENC[:], in0=WE[:, :Q], in1=WE[:, Q:], op=op.mult)
                IU = sb.tile([128, 2 * Q], DT, tag=f"IU_{p}")   # [-iou | -ue]
                RC = sb.tile([128, 2 * Q], DT, tag=f"RC_{p}")   # [1/-union | 1/enc]
                nc.vector.reciprocal(out=RC[:, :Q], in_=T1[:, Q:])
                nc.vector.reciprocal(out=RC[:, Q:], in_=ENC[:])
                nc.vector.tensor_tensor(out=IU[:, :Q], in0=T1[:, :Q], in1=RC[:, :Q], op=op.mult)
                nc.gpsimd.tensor_tensor(out=IU[:, Q:], in0=T1[:, Q:], in1=RC[:, Q:], op=op.mult)
                P1 = sb.tile([128, Q], DT, tag=f"P1_{p}")
                ve.tensor_tensor(out=P1[:], in0=IU[:, :Q], in1=IU[:, Q:], op=op.add)
                P2 = sb.tile([128, Q], DT, tag=f"P2_{p}")
                tt_eng.tensor_tensor(out=P2[:], in0=LB[:, :Q], in1=LB[:, Q:], op=op.add)
                P3 = sb.tile([128, Q], DT, tag=f"P3_{p}")
                ve.tensor_tensor(out=P3[:], in0=P2[:], in1=FA[:, Q:], op=op.add)
                OUT = sb.tile([128, Q], DT, tag=f"OUT_{p}")
                tt_eng.tensor_tensor(out=OUT[:], in0=P3[:], in1=P1[:], op=op.add)
                (nc.sync if p % 2 == 0 else nc.scalar).dma_start(out=cost_o[p], in_=OUT[:])

    _split_wide_waits(nc, mybir)
    return nc


def _lsa(cost):
    # Hungarian (shortest augmenting path), identical algorithm to reference.
    cost = np.asarray(cost, dtype=np.float64)
    n, m = cost.shape
    u = np.zeros(n + 1)
    v = np.zeros(m + 1)
    p = np.zeros(m + 1, dtype=np.int64)
    way = np.zeros(m + 1, dtype=np.int64)
    for i in range(1, n + 1):
        p[0] = i
        j0 = 0
        minv = np.full(m + 1, np.inf)
        used = np.zeros(m + 1, dtype=bool)
        while True:
            used[j0] = True
            i0 = p[j0]
            cur = cost[i0 - 1, :] - u[i0] - v[1:]
            free = ~used[1:]
            upd = free & (cur < minv[1:])
            minv[1:][upd] = cur[upd]
            way[1:][upd] = j0
            cand = np.where(free, minv[1:], np.inf)
            j1 = int(np.argmin(cand)) + 1
            delta = cand[j1 - 1]
            u[p[used]] += delta
            v[used] -= delta
            minv[~used] -= delta
            j0 = j1
            if p[j0] == 0:
                break
        while j0:
            j1 = way[j0]
            p[j0] = p[j1]
            j0 = j1
    ans = np.zeros(n, dtype=np.int64)
    for j in range(1, m + 1):
        if p[j] > 0:
            ans[p[j] - 1] = j - 1
    return ans


def _host_prep(logits, pred_bbox, target_bbox):
    import ml_dtypes
    logits = np.ascontiguousarray(logits, np.float32)
    pb = np.ascontiguousarray(pred_bbox, np.float32)
    tb = np.ascontiguousarray(target_bbox, np.float32)

    pcx, pcy, pw, ph = pb[..., 0], pb[..., 1], pb[..., 2], pb[..., 3]
    px1, py1 = pcx - 0.5 * pw, pcy - 0.5 * ph
    px2, py2 = pcx + 0.5 * pw, pcy + 0.5 * ph
    area1 = pw * ph
    dl = (logits[..., 1] - logits[..., 0]).astype(np.float64)
    f = (1.0 / (1.0 + np.exp(-dl))).astype(np.float32)   # 1 - p0 = sigmoid(l1-l0)
    # [B, 10, Q], quantity-major
    qr_all = np.stack([px1, -px2, py1, -py2, pcx, pcy, pw, ph, area1, f], axis=1)

    tcx, tcy, tw, th = tb[..., 0], tb[..., 1], tb[..., 2], tb[..., 3]
    tx1, ty1 = tcx - 0.5 * tw, tcy - 0.5 * th
    tx2, ty2 = tcx + 0.5 * tw, tcy + 0.5 * th
    area2 = tw * th
    # [B, T, 11]
    tr_all = np.stack([tw, th, area2, -tx1, tx2, -ty1, ty2, -tcx, -tcy, -tw, -th],
                      axis=-1)

    in_maps = []
    for c in range(N_CORES):
        i0 = c * IMGS_PER_CORE
        # qrows: [pair, group(3) x img(2), 4*Q] bf16, groups of quantities
        qc4 = qr_all[i0:i0 + IMGS_PER_CORE].reshape(PAIRS_PER_CORE, 2, 10, Q)
        # pre-round to bf16 so the A-B row is an exact difference of bf16 values
        qc4 = qc4.astype(ml_dtypes.bfloat16).astype(np.float32)
        qc = np.zeros((PAIRS_PER_CORE, 96, 4 * Q), np.float32)
        for g, ks in enumerate(([0, 1, 2, 3], [4, 5, 6], [7, 8, 9])):
            for j, k in enumerate(ks):
                # row0 = imgA - imgB (selected on partitions 0-63), row1 = imgB
                qc[:, g * 32 + 0, j * Q:(j + 1) * Q] = qc4[:, 0, k, :] - qc4[:, 1, k, :]
                qc[:, g * 32 + 1, j * Q:(j + 1) * Q] = qc4[:, 1, k, :]
        # trows: [128 partitions, pair*11]
        tc_ = tr_all[i0:i0 + IMGS_PER_CORE].reshape(PAIRS_PER_CORE, 128, 11)
        tc_ = tc_.transpose(1, 0, 2).reshape(128, PAIRS_PER_CORE * 11)
        in_maps.append({
            "qrows": np.ascontiguousarray(qc).astype(ml_dtypes.bfloat16),
            "trows": np.ascontiguousarray(tc_),
        })
    return in_maps


def _finalize(logits, pred_bbox, target_bbox, target_labels, src):
    labels = np.asarray(target_labels).astype(np.int64)
    lg = np.asarray(logits, np.float64)
    pb = np.asarray(pred_bbox, np.float64)
    tb = np.asarray(target_bbox, np.float64)
    bidx = np.arange(B)[:, None]

    # CE pieces (exact, host): nlpk = -logp_k
    dl = lg[..., 1] - lg[..., 0]
    nlp1 = np.logaddexp(0.0, -dl)       # -logp1 = softplus(l0-l1)
    nlp0 = np.logaddexp(0.0, dl)        # -logp0 = softplus(l1-l0)
    g = nlp0 - CLS_SCALE * nlp1         # matched-query correction (labels are 0)
    A = nlp1.sum()
    w = np.ones(C); w[-1] = CLS_SCALE
    wt_sum = CLS_SCALE * (B * Q) + np.sum(w[labels] - CLS_SCALE)
    ce = (CLS_SCALE * A + g[bidx, src].sum()) / wt_sum

    mp = pb[bidx, src].reshape(-1, 4)
    mt = tb.reshape(-1, 4)
    nb = B * T
    l1 = np.abs(mp - mt).sum() / nb

    def corners(x):
        cx, cy, ww, hh = x[:, 0], x[:, 1], x[:, 2], x[:, 3]
        return np.stack([cx - .5 * ww, cy - .5 * hh, cx + .5 * ww, cy + .5 * hh], -1)

    c1, c2 = corners(mp), corners(mt)
    a1 = (c1[:, 2] - c1[:, 0]) * (c1[:, 3] - c1[:, 1])
    a2 = (c2[:, 2] - c2[:, 0]) * (c2[:, 3] - c2[:, 1])
    lt = np.maximum(c1[:, :2], c2[:, :2]); rb = np.minimum(c1[:, 2:], c2[:, 2:])
    wh = np.clip(rb - lt, 0, None); inter = wh[:, 0] * wh[:, 1]
    union = a1 + a2 - inter
    iou = inter / union
    lte = np.minimum(c1[:, :2], c2[:, :2]); rbe = np.maximum(c1[:, 2:], c2[:, 2:])
    whe = np.clip(rbe - lte, 0, None); encl = whe[:, 0] * whe[:, 1]
    giou = iou - (encl - union) / encl
    lgi = (1.0 - giou).sum() / nb
    return ce + BBOX_SCALE * l1 + GIOU_SCALE * lgi


def kernel(logits, pred_bbox, target_bbox, target_labels):
    import os
    os.environ["BASS_NEVER_TRACE"] = "1"   # no NTFF hook in this container
    from concourse.bass_utils import run_bass_kernel_spmd

    if "nc" not in _CACHE:
        _CACHE["nc"] = _build_program()
    nc = _CACHE["nc"]

    in_maps = _host_prep(logits, pred_bbox, target_bbox)
    res = run_bass_kernel_spmd(nc, in_maps, core_ids=list(range(N_CORES)))
    _CACHE["last_res"] = res

    cost_T = np.zeros((B, T, Q), np.float32)   # [img, target, query]
    for c in range(N_CORES):
        cb = np.asarray(res.results[c]["cost"]).astype(np.float32).reshape(PAIRS_PER_CORE, 2, 64, Q)
        i0 = c * IMGS_PER_CORE
        for p in range(PAIRS_PER_CORE):
            cost_T[i0 + 2 * p] = cb[p, 0]
            cost_T[i0 + 2 * p + 1] = cb[p, 1]

    src = np.zeros((B, T), np.int64)
    for i in range(B):
        src[i] = _lsa(cost_T[i])

    total = _finalize(logits, pred_bbox, target_bbox, target_labels, src)
    return np.float32(total)
